# revision 1
# baseline (speedup 1.0000x reference)
"""Trainium2 Bass kernel for the attention layer:

    f = wf@x+bf; g = wg@x+bg; h = wh@x+bh            (1x1 convs, Ci=32)
    attn = softmax(f^T g, axis=-1)                   (per batch, N=4096)
    out = (wv @ (h @ attn^T) + bv) * gamma + x

Sharding: 8 cores = 4 batches x 2 query-halves (2048 queries each).
Each core receives the full (256, 4096) batch slice with its query half
permuted to the front, so the SPMD program uses fixed offsets.

Per-core dataflow (matmuls fp32r, PSUM fp32 accumulate):
  - warm-up: a dense block of dummy matmuls at t=0 so the PE HAM clock
    gate reaches 8/8 before the real work, plus a dummy exp to pull the
    ACT table load forward.
  - f/g are computed replicated onto 4 partition strips (host-replicated
    wf^T/wg^T with M=128), so the K=32 logits matmuls can be row-packed
    with tile_position: consecutive key chunks run concurrently in PE
    row bands, each writing its own PSUM bank.
  - hT (4096, 32) k-major blocks: lhsT=x k-chunk, rhs=wh^T.
  - per 512-query chunk: 32 k-chunk matmuls logitsT = g^T f (k on
    partitions, row-packed) -> ACT exp PSUM->SBUF (1024 wide) -> 32
    k-chunk accumulation rounds, each round two CONCURRENT column-tiled
    matmuls into one PSUM bank: rows 0-31 accumulate the softmax
    denominator (ones stationary), rows 32-63 accumulate x0 = h@attn^T
    (hT stationary). Reciprocal of row 0, GPSIMD partition-broadcast,
    multiply -> x0a; project with wv*gamma; bias (bv+wv@bh folded on
    host) + residual fused in one scalar_tensor_tensor; DMA out.
"""

import os
import numpy as np
import ml_dtypes

import concourse.bass as bass
import concourse.mybir as mybir
import concourse.tile as tile
from concourse import bacc
from concourse.bass import ts
from concourse.bass_utils import run_bass_kernel_spmd

F32 = mybir.dt.float32
F32R = mybir.dt.float32r
BF16 = mybir.dt.bfloat16
EXP = mybir.ActivationFunctionType.Exp
ADD = mybir.AluOpType.add

B, C, W, H = 4, 256, 64, 64
N = W * H            # 4096 keys/queries per batch
CI = 32              # inner channels
NCORES = 8
NQ = N // 2          # queries per core
QC = 512             # query chunk = one fp32 PSUM bank
NQC = NQ // QC       # 4 query chunks per core
KC = 128             # key chunk = partition dim
NKC = N // KC        # 32 key chunks
GRP = 2              # key chunks per ACT exp group (PSUM banks per tile)
NWARM = 8            # dummy fp32 matmuls to warm the PE clock gate

# Trace knob for test harnesses: set kernel.TRACE = True to profile.
TRACE = False
LAST_EXEC_NS = None

_cached_nc = None


def _mm(nc, out, lhsT, rhs, start, stop, tile_position=None):
    nc.tensor.matmul(out, lhsT=lhsT, rhs=rhs, start=start, stop=stop,
                     tile_position=tile_position)


def _build():
    nc = bacc.Bacc(
        "TRN2", target_bir_lowering=False, debug=False, num_devices=NCORES
    )
    x_d = nc.dram_tensor("x", (C, N), F32R, kind="ExternalInput").ap()
    wfT_d = nc.dram_tensor("wfT", (C, 128), F32R, kind="ExternalInput").ap()
    wgT_d = nc.dram_tensor("wgT", (C, 128), F32R, kind="ExternalInput").ap()
    whT_d = nc.dram_tensor("whT", (C, CI), BF16, kind="ExternalInput").ap()
    xbf_d = nc.dram_tensor("xbf", (C, N), BF16, kind="ExternalInput").ap()
    wvT_d = nc.dram_tensor("wvT", (CI + 1, C), F32R, kind="ExternalInput").ap()
    bf_d = nc.dram_tensor("bf", (128, 1), F32, kind="ExternalInput").ap()
    bg_d = nc.dram_tensor("bg", (128, 1), F32, kind="ExternalInput").ap()
    out_d = nc.dram_tensor("out", (C, NQ), F32, kind="ExternalOutput").ap()

    xr = x_d.rearrange("(cc p) n -> p cc n", p=128)
    outr = out_d.rearrange("(oc p) n -> p oc n", p=128)

    with tile.TileContext(nc) as tc:
        with (
            tc.tile_pool(name="consts", bufs=1) as consts,
            tc.tile_pool(name="data", bufs=1) as data,
            tc.tile_pool(name="eTp", bufs=6) as eTp,
            tc.tile_pool(name="smallp", bufs=2) as smallp,
            tc.tile_pool(name="outp", bufs=3) as outp,
            tc.tile_pool(name="pl", bufs=2, space="PSUM") as pl,
            tc.tile_pool(name="pp", bufs=2, space="PSUM") as pp,
            tc.tile_pool(name="px0", bufs=2, space="PSUM") as px0,
        ):
            # ---- PE + ACT warm-up (overlaps the input DMAs) ----
            scratch = consts.tile([128, QC], F32)
            nc.vector.memset(scratch, 0.0)
            wps = pp.tile([128, QC], F32, tag="pp")
            for i in range(NWARM):
                nc.tensor.matmul(
                    wps, lhsT=scratch[:, 0:128], rhs=scratch,
                    start=True, stop=True, skip_group_check=True,
                )
            scratch2 = consts.tile([1, 8], F32)
            nc.scalar.activation(
                out=scratch2, in_=scratch[0:1, 0:8], func=EXP
            )

            # ---- constants ----
            wfT_sb = consts.tile([128, 2, 128], F32R)
            nc.sync.dma_start(
                out=wfT_sb, in_=wfT_d.rearrange("(cc p) o -> p cc o", p=128)
            )
            wgT_sb = consts.tile([128, 2, 128], F32R)
            nc.sync.dma_start(
                out=wgT_sb, in_=wgT_d.rearrange("(cc p) o -> p cc o", p=128)
            )
            whT_sb = consts.tile([128, 2, CI], BF16)
            nc.sync.dma_start(
                out=whT_sb, in_=whT_d.rearrange("(cc p) o -> p cc o", p=128)
            )
            wvT_sb = consts.tile([CI + 1, 2, 128], F32R)
            nc.sync.dma_start(
                out=wvT_sb, in_=wvT_d.rearrange("p (oc m) -> p oc m", oc=2)
            )
            bf_sb = consts.tile([128, 1], F32)
            nc.sync.dma_start(out=bf_sb, in_=bf_d)
            bg_sb = consts.tile([128, 1], F32)
            nc.sync.dma_start(out=bg_sb, in_=bg_d)
            ones_sb = consts.tile([128, 1], F32)
            nc.vector.memset(ones_sb, 1.0)
            scratchR = consts.tile([128, QC], F32R)
            nc.vector.tensor_copy(scratchR, scratch)

            # ---- x (fp32 for f/g/residual, bf16 for the hT matmuls) ----
            x_sb = data.tile([128, 2, N], F32R)
            xbf_sb = data.tile([128, 2, N], BF16)
            xbfr = xbf_d.rearrange("(cc p) n -> p cc n", p=128)
            for s in range(4):
                nc.sync.dma_start(
                    out=x_sb[:, :, ts(s, N // 4)], in_=xr[:, :, ts(s, N // 4)]
                )
                nc.sync.dma_start(
                    out=xbf_sb[:, :, ts(s, N // 4)],
                    in_=xbfr[:, :, ts(s, N // 4)],
                )

            # ---- f, g (replicated on 4 strips), hT ----
            f_sb = data.tile([128, NQ], F32R)
            g_sb = data.tile([128, N], F32R)
            hT_sb = data.tile([128, NKC, CI + 1], F32R)
            nc.vector.tensor_copy(
                hT_sb[:, :, 0:1], ones_sb.to_broadcast([128, NKC, 1])
            )

            def emit_f(j):
                ps = pp.tile([128, QC], F32, tag="pp", name=f"psf{j}")
                for cc in range(2):
                    _mm(nc, ps, wfT_sb[:, cc, :],
                        x_sb[:, cc, ts(j, QC)], cc == 0, cc == 1)
                nc.vector.tensor_scalar_add(
                    f_sb[:, ts(j, QC)], ps, bf_sb
                )

            def emit_g(j):
                ps = pp.tile([128, QC], F32, tag="pp", name=f"psg{j}")
                for cc in range(2):
                    _mm(nc, ps, wgT_sb[:, cc, :],
                        x_sb[:, cc, ts(j, QC)], cc == 0, cc == 1)
                nc.vector.tensor_scalar_add(
                    g_sb[:, ts(j, QC)], ps, bg_sb
                )

            def emit_hT(kc):
                ps = pp.tile([128, QC], F32, tag="pp", name=f"psh{kc}")
                for cc in range(2):
                    _mm(nc, ps[:, 0:CI], xbf_sb[:, cc, ts(kc, KC)],
                        whT_sb[:, cc, :], cc == 0, cc == 1)
                nc.vector.tensor_copy(hT_sb[:, kc, 1 : CI + 1], ps[:, 0:CI])

            # f/g/hT are emitted just-in-time inside chunk 0's group
            # loop below, so the PE's in-order stream interleaves them
            # with chunk 0's logits/x0 work instead of running the whole
            # phase serially up front.
            emit_f(0)

            # ---- main loop over query chunks ----
            for qi in range(NQC):
                # row 0: softmax denominator (ones column in hT);
                # rows 1-32: x0 channels.
                x0 = px0.tile([CI + 1, QC], F32)
                x0q = []
                for g0 in range(0, NKC, GRP):
                    if qi == 0:
                        if g0 % 4 == 0:
                            emit_g(g0 // 4)
                        for kc in range(g0, g0 + GRP):
                            emit_hT(kc)
                    ps = pl.tile([128, GRP, QC], F32, tag="lg")
                    eT = eTp.tile([128, GRP, QC], F32R)
                    for j in range(GRP):
                        kc = g0 + j
                        # row-packed: strip kc%4 holds its own copy of
                        # g/f, so adjacent matmuls execute concurrently
                        # in different PE row bands.
                        s = kc % 4
                        sl = slice(32 * s, 32 * (s + 1))
                        nc.tensor.matmul(
                            ps[:, j, :],
                            lhsT=g_sb[sl, ts(kc, KC)],
                            rhs=f_sb[sl, ts(qi, QC)],
                            start=True, stop=True,
                            tile_position=(32 * s, 0),
                        )
                    nc.scalar.activation(
                        out=eT[:, :, :], in_=ps[:, :, :], func=EXP
                    )
                    # software-pipeline the x0 stage by one group: its
                    # wait on this group's exp then overlaps the NEXT
                    # group's logits in the in-order PE stream.
                    x0q.append((g0, eT))
                    if len(x0q) > 2:
                        pg0, peT = x0q.pop(0)
                        for j in range(GRP):
                            kc = pg0 + j
                            _mm(nc, x0, hT_sb[:, kc, :], peT[:, j, :],
                                kc == 0, kc == NKC - 1)
                for pg0, peT in x0q:
                    for j in range(GRP):
                        kc = pg0 + j
                        _mm(nc, x0, hT_sb[:, kc, :], peT[:, j, :],
                            kc == 0, kc == NKC - 1)
                if qi == 0:
                    for j in range(1, NQ // QC):
                        emit_f(j)
                # softmax divide: row 0 of x0 is the denominator
                rcp = smallp.tile([1, QC], F32, tag="rcp")
                nc.vector.reciprocal(rcp, x0[0:1, :])
                rcp_b = smallp.tile([CI + 1, QC], F32, tag="rcpb")
                nc.gpsimd.partition_broadcast(rcp_b, rcp)
                x0a = smallp.tile([CI + 1, QC], F32R, tag="x0a")
                nc.vector.tensor_mul(x0a, x0, rcp_b)
                # project back to C channels; bias + residual fused
                for oc in range(2):
                    vps = pp.tile([128, QC], F32, tag="pp")
                    _mm(nc, vps, wvT_sb[:, oc, :], x0a, True, True)
                    ot = outp.tile([128, QC], F32)
                    nc.vector.tensor_add(
                        ot, vps, x_sb[:, oc, ts(qi, QC)].bitcast(F32)
                    )
                    nc.sync.dma_start(out=outr[:, oc, ts(qi, QC)], in_=ot)
                if qi < NQC - 1:
                    # dense fp32r dummy matmuls: re-warm the PE clock
                    # gate in case a stall re-throttled it this chunk.
                    wb = pp.tile([128, QC], F32, tag="pp")
                    for i in range(6):
                        nc.tensor.matmul(
                            wb, lhsT=scratchR[:, 0:128], rhs=scratchR,
                            start=True, stop=True, skip_group_check=True,
                        )

    nc.compile()
    return nc


def kernel(x, wf, bf, wg, bg, wh, bh, wv, bv, gamma):
    global _cached_nc, LAST_EXEC_NS
    if _cached_nc is None:
        _cached_nc = _build()
    nc = _cached_nc

    x = np.asarray(x, dtype=np.float32)
    wf = np.asarray(wf, dtype=np.float32)
    bf = np.asarray(bf, dtype=np.float32)
    wg = np.asarray(wg, dtype=np.float32)
    bg = np.asarray(bg, dtype=np.float32)
    wh = np.asarray(wh, dtype=np.float32)
    bh = np.asarray(bh, dtype=np.float32)
    wv = np.asarray(wv, dtype=np.float32)
    bv = np.asarray(bv, dtype=np.float32)
    g0 = float(np.asarray(gamma, dtype=np.float32).reshape(-1)[0])

    xf = np.ascontiguousarray(x.reshape(B, C, N))
    # f/g weights replicated 4x along M so f/g land replicated on the
    # four 32-partition strips (enables row-packed logits matmuls).
    wfT = np.ascontiguousarray(np.tile(wf.T, (1, 4)))     # (256, 128)
    wgT = np.ascontiguousarray(np.tile(wg.T, (1, 4)))     # (256, 128)
    whT = np.ascontiguousarray(wh.T.astype(ml_dtypes.bfloat16))
    wvT = np.empty((CI + 1, C), np.float32)               # aug: bias row 0
    wvT[0, :] = g0 * (bv + wv @ bh)
    wvT[1:, :] = g0 * wv.T
    bf4 = np.ascontiguousarray(np.tile(bf, 4).reshape(128, 1))
    bg4 = np.ascontiguousarray(np.tile(bg, 4).reshape(128, 1))

    in_maps = []
    for core in range(NCORES):
        b, half = divmod(core, 2)
        xb = xf[b]
        if half:
            xb = np.ascontiguousarray(
                np.concatenate([xb[:, NQ:], xb[:, :NQ]], axis=1)
            )
        in_maps.append(
            {"x": xb, "xbf": xb.astype(ml_dtypes.bfloat16), "wfT": wfT,
             "wgT": wgT, "whT": whT, "wvT": wvT, "bf": bf4, "bg": bg4}
        )

    res = run_bass_kernel_spmd(
        nc, in_maps, list(range(NCORES)),
        trace=TRACE or bool(os.environ.get("BASS_KERNEL_TRACE")),
    )
    LAST_EXEC_NS = res.exec_time_ns

    out = np.empty((B, C, N), np.float32)
    for core in range(NCORES):
        b, half = divmod(core, 2)
        out[b][:, half * NQ : (half + 1) * NQ] = res.results[core]["out"]
    return out.reshape(B, C, W, H)



# revision 15
# speedup vs baseline: 1.2520x; 1.2520x over previous
"""Trainium2 Bass kernel for the attention layer:

    f = wf@x+bf; g = wg@x+bg; h = wh@x+bh            (1x1 convs, Ci=32)
    attn = softmax(f^T g, axis=-1)                   (per batch, N=4096)
    out = (wv @ (h @ attn^T) + bv) * gamma + x

Sharding: 8 cores = 4 batches x 2 query-halves (2048 queries each).
Each core receives the full (256, 4096) batch slice with its query half
permuted to the front, so the SPMD program uses fixed offsets.

Key design points (vs. the fp32r baseline):
  - logits are reassociated: f^T g = x_q^T (wf^T wg) x_k. A = wf^T wg is
    a weight-only 256x256 matrix folded on host, so the contraction K
    becomes 256 and maps onto fp8 DoubleRow matmuls (2 k-tiles of 128,
    0.5 cycles/row): g' = A x on device (8 DoubleRow matmuls), then
    logitsT tiles [128k x 512q] at 256 cycles each.  f and g are never
    materialized; the bf/bg bias terms reduce to a per-key logit offset
    u_k = (bf@wg)@x_k (query-side terms cancel in softmax), computed on
    host and folded into the exp bias.
  - x0 = h @ attn^T accumulates in fp8 DoubleRow as well: hT (keys x
    [ones|h]) quantized to e5m2, exp values stored as e5m2.
  - exp is the real bottleneck (~8.4M elements/core, ACT-only would be
    66us), so it is split across THREE engines: ACT does true
    exp->e5m2 (bias AP = per-key offset), while DVE and GPSIMD compute
    the e5m2 BIT PATTERN directly with one affine op each
    (Schraudolph: uint8_saturate(round(5.77*(logit+u-16) + 60)) is the
    e5m2 encoding of ~exp(logit+u-16)).  Work is assigned greedily by
    a static cost model.  The softmax denominator rides along as a
    "ones" column in hT (row 0 of the x0 PSUM tile).
  - residual + projection: v = wv' x0a accumulates in PSUM, then the
    residual x is ADDED BY THE PE (identity matmul accumulate), so the
    vector engines only do one PSUM->SBUF copy per output tile.
  - all global scaling (A*256, h*64, gamma, 1/denominator) is folded
    into host-side weight prep / the exp affine / wv'.
"""

import math
import os
import numpy as np
import ml_dtypes

import concourse.bass as bass
import concourse.mybir as mybir
import concourse.tile as tile
from concourse import bacc
from concourse.bass import ts
from concourse.bass_utils import run_bass_kernel_spmd

F32 = mybir.dt.float32
F32R = mybir.dt.float32r
U8 = mybir.dt.uint8
F8E4 = mybir.dt.float8e4
F8E5 = mybir.dt.float8e5
DR = mybir.MatmulPerfMode.DoubleRow
EXPF = mybir.ActivationFunctionType.Exp
ADD = mybir.AluOpType.add
MULT = mybir.AluOpType.mult

B, C, W, H = 4, 256, 64, 64
N = W * H            # 4096 keys per batch
CI = 32              # inner channels
NCORES = 8
NQ = N // 2          # queries per core
QC = 512             # query chunk (one fp32 PSUM bank)
QP = 2 * QC          # query pair chunk (exp tile width)
NQP = NQ // QP       # 2 query-pair iterations per core
KC = 128             # key chunk = partition dim
NKC = N // KC        # 32 key chunks
MH = 34              # x0 partitions: [denominator ones | 32 h | pad]
MHP = 48             # padded k-tile stride: DoubleRow needs 16B alignment
NWARM = 8
RUNWAY = 2           # x0 pairs lag behind logits by this many pairs
LOGITS_DR = True     # debug toggle: DoubleRow for the logits matmuls
X0_DR = True         # debug toggle: DoubleRow for the x0 matmuls
CONST_BIAS = False   # debug toggle: use float exp biases (assumes bf==0)

SA = 128.0           # A (logits weight) scale: keeps |g'| < 240
SH = 64.0            # h scale
CSHIFT = 16.0        # global exp shift (softmax-invariant)
L2E4 = 4.0 * math.log2(math.e)            # 5.7708: e5m2 codes per nat
AEXP_SCH = L2E4 / SA                      # Schraudolph scale on psum
AEXP_NAT = 1.0 / SA                       # true-exp scale on psum

TRACE = False
DEBUG = False
LAST_EXEC_NS = None
LAST_RES = None

_cached_nc = None


class Sched:
    """Greedy static load balancer for the three elementwise engines."""

    # (ns per free-dim element, fixed ns per instruction).  GPSIMD has no
    # PSUM port (BIR verifier rejects Pool-engine PSUM operands), so all
    # PSUM-sourced elementwise work must go to ACT or DVE.
    COST = {"act": (0.87, 240), "dve": (1.08, 250), "gp": (2.05, 170)}

    def __init__(self):
        self.load = {"act": 0.0, "dve": 0.0, "gp": 0.0}

    def pick(self, nfree, allowed=("act", "dve")):
        def fin(e):
            v, f = self.COST[e]
            return self.load[e] + v * nfree + f
        best = min(allowed, key=fin)
        self.load[best] = fin(best)
        return best

    def charge(self, eng, nfree):
        v, f = self.COST[eng]
        self.load[eng] += v * nfree + f


def _build():
    nc = bacc.Bacc(
        "TRN2", target_bir_lowering=False, debug=False, num_devices=NCORES
    )
    x8_d = nc.dram_tensor("x8", (C, N), U8, kind="ExternalInput").ap()
    x32_d = nc.dram_tensor("x32", (C, N), F32R, kind="ExternalInput").ap()
    ucs_d = nc.dram_tensor("ucs", (128, NKC), F32, kind="ExternalInput").ap()
    ucn_d = nc.dram_tensor("ucn", (128, NKC), F32, kind="ExternalInput").ap()
    A8_d = nc.dram_tensor("A8", (128, 512), U8, kind="ExternalInput").ap()
    whu8_d = nc.dram_tensor("whu8", (128, 2 * MHP), U8, kind="ExternalInput").ap()
    wvp_d = nc.dram_tensor("wvp", (MH, C), F32R, kind="ExternalInput").ap()
    ident_d = nc.dram_tensor("ident", (128, 128), F32R, kind="ExternalInput").ap()
    out_d = nc.dram_tensor("out", (C, NQ), F32, kind="ExternalOutput").ap()
    if DEBUG:
        dbg_g8_d = nc.dram_tensor("dbg_g8", (128, 2 * N), U8, kind="ExternalOutput").ap()
        dbg_hT_d = nc.dram_tensor("dbg_hT", (128, NKC * MHP), U8, kind="ExternalOutput").ap()
        dbg_eT_d = [
            nc.dram_tensor(f"dbg_eT{i}", (128, 2 * QP), U8, kind="ExternalOutput").ap()
            for i in range(4)
        ]
        dbg_x0a_d = nc.dram_tensor("dbg_x0a", (MH, QC), F32R, kind="ExternalOutput").ap()
        dbg_rcp_d = nc.dram_tensor("dbg_rcp", (1, QC), F32, kind="ExternalOutput").ap()
        dbg_uc_d = nc.dram_tensor("dbg_uc", (128, 2 * NKC), F32, kind="ExternalOutput").ap()
        dbg_ps_d = [
            nc.dram_tensor(f"dbg_ps{i}", (128, QP), F32, kind="ExternalOutput").ap()
            for i in range(4)
        ]

    x8r = x8_d.rearrange("(cc p) n -> p cc n", p=128)
    x32r = x32_d.rearrange("(cc p) n -> p cc n", p=128)
    outr = out_d.rearrange("(oc p) n -> p oc n", p=128)

    sched = Sched()

    with tile.TileContext(nc) as tc:
        with (
            tc.tile_pool(name="consts", bufs=1) as consts,
            tc.tile_pool(name="data", bufs=1) as data,
            tc.tile_pool(name="eTp", bufs=6) as eTp,
            tc.tile_pool(name="smallp", bufs=2) as smallp,
            tc.tile_pool(name="outp", bufs=3) as outp,
            tc.tile_pool(name="pl", bufs=3, space="PSUM") as pl,
            tc.tile_pool(name="px0", bufs=2, space="PSUM") as px0,
        ):
            # ---- PE warm-up (overlaps the input DMAs) ----
            scratch = consts.tile([128, QC], F32)
            nc.vector.memset(scratch, 0.0)
            wps = px0.tile([128, QC // 2], F32, tag="x0", name="warm")
            for i in range(NWARM):
                nc.tensor.matmul(
                    wps, lhsT=scratch[:, 0:128].bitcast(F32R),
                    rhs=scratch[:, 0:256].bitcast(F32R),
                    start=True, stop=True, skip_group_check=True,
                )
            # absorb the ACT exp-table load before the main loop
            scratch2 = consts.tile([1, 8], F8E5)
            nc.scalar.activation(
                out=scratch2, in_=scratch[0:1, 0:8].bitcast(F32), func=EXPF
            )

            # ---- constants ----
            A8_sb = consts.tile([128, 2, 2, 128], U8)
            nc.sync.dma_start(
                out=A8_sb, in_=A8_d.rearrange("p (cc o m) -> p cc o m", cc=2, o=2)
            )
            whu8_sb = consts.tile([128, 2, MHP], U8)
            nc.sync.dma_start(
                out=whu8_sb, in_=whu8_d.rearrange("p (cc m) -> p cc m", cc=2)
            )
            wvp_sb = consts.tile([MH, 2, 128], F32R)
            nc.sync.dma_start(
                out=wvp_sb, in_=wvp_d.rearrange("p (oc m) -> p oc m", oc=2)
            )
            ident_sb = consts.tile([128, 128], F32R)
            nc.sync.dma_start(out=ident_sb, in_=ident_d)
            ucs_sb = consts.tile([128, NKC], F32)
            nc.sync.dma_start(out=ucs_sb, in_=ucs_d)
            ucn_sb = consts.tile([128, NKC], F32)
            nc.sync.dma_start(out=ucn_sb, in_=ucn_d)
            cbias_n = consts.tile([128, 1], F32)
            nc.vector.memset(cbias_n, -CSHIFT)
            cbias_s = consts.tile([128, 1], F32)
            nc.vector.memset(cbias_s, L2E4 * -CSHIFT + 60.0)

            # ---- x input: fp8 for matmuls, f32r for the residual ----
            x8_sb = data.tile([128, 2, N], U8)
            x32_sb = data.tile([128, 2, N], F32R)
            for s in range(4):
                nc.sync.dma_start(
                    out=x8_sb[:, :, ts(s, N // 4)], in_=x8r[:, :, ts(s, N // 4)]
                )
            for s in range(4):
                nc.sync.dma_start(
                    out=x32_sb[:, :, ts(s, N // 4)], in_=x32r[:, :, ts(s, N // 4)]
                )

            g8_sb = data.tile([128, 2, N], F8E4)
            hT8_sb = data.tile([128, NKC, MHP], F8E5)
            nc.vector.memset(hT8_sb, 0.0)
            nc.vector.memset(hT8_sb[:, :, 0:1], 1.0)

            def ew_copy(dst, src, nfree):
                e = sched.pick(nfree)
                if e == "act":
                    nc.scalar.copy(dst, src)
                else:
                    nc.vector.tensor_copy(dst, src)

            # ---- hT (keys x [ones|64h|pad]) in e5m2, 8 key chunks/bank ----
            for b8 in range(4):
                ph = px0.tile([128, 8, MHP], F32, tag="x0", name=f"ph{b8}")
                for j in range(8):
                    kc = b8 * 8 + j
                    nc.tensor.matmul(
                        ph[:, j, :],
                        lhsT=x8_sb[:, :, ts(kc, KC)].bitcast(F8E4),
                        rhs=whu8_sb.bitcast(F8E4),
                        start=True, stop=True, perf_mode=DR,
                    )
                ew_copy(
                    hT8_sb[:, b8 * 8 : b8 * 8 + 8, 1 : CI + 1], ph[:, :, 0:CI],
                    8 * CI,
                )

            # ---- g' = A x (256 x N) in e4m3 ----
            for ch in range(8):
                pg = pl.tile([128, 2, QC], F32, tag="lg", name=f"g{ch}")
                for o in range(2):
                    nc.tensor.matmul(
                        pg[:, o, :],
                        lhsT=A8_sb[:, :, o, :].bitcast(F8E4),
                        rhs=x8_sb[:, :, ts(ch, QC)].bitcast(F8E4),
                        start=True, stop=True, perf_mode=DR,
                    )
                ew_copy(g8_sb[:, :, ts(ch, QC)], pg, QP)

            if DEBUG:
                nc.sync.dma_start(out=dbg_uc_d[:, 0:NKC], in_=ucs_sb)
                nc.sync.dma_start(out=dbg_uc_d[:, NKC : 2 * NKC], in_=ucn_sb)
                nc.sync.dma_start(
                    out=dbg_g8_d, in_=g8_sb.bitcast(U8).rearrange("p a b -> p (a b)")
                )
                nc.sync.dma_start(
                    out=dbg_hT_d, in_=hT8_sb.bitcast(U8).rearrange("p a b -> p (a b)")
                )

            # ---- main loop over query pairs ----
            pend_v = []  # deferred projection/residual work

            def flush_v():
                while pend_v:
                    qg, x0t = pend_v.pop(0)
                    rcp = smallp.tile([1, QC], F32, tag="rcp", name=f"r{qg}")
                    nc.vector.reciprocal(rcp, x0t[0:1, :])
                    rcp_b = smallp.tile([MH, QC], F32, tag="rcpb", name=f"rb{qg}")
                    nc.gpsimd.partition_broadcast(rcp_b, rcp)
                    sched.charge("dve", QC)
                    sched.charge("gp", QC)
                    x0a = smallp.tile([MH, QC], F32R, tag="x0a", name=f"xa{qg}")
                    nc.vector.tensor_mul(x0a, x0t[0:MH, :], rcp_b)
                    if DEBUG and qg == 0:
                        nc.sync.dma_start(out=dbg_x0a_d, in_=x0a)
                        nc.sync.dma_start(out=dbg_rcp_d, in_=rcp)
                    sched.charge("dve", QC)
                    vt = pl.tile([128, 2, QC], F32, tag="lg", name=f"v{qg}")
                    for oc in range(2):
                        nc.tensor.matmul(
                            vt[:, oc, :], lhsT=wvp_sb[:, oc, :], rhs=x0a,
                            start=True, stop=False,
                        )
                        nc.tensor.matmul(
                            vt[:, oc, :], lhsT=ident_sb,
                            rhs=x32_sb[:, oc, ts(qg, QC)],
                            start=False, stop=True,
                        )
                    ot = outp.tile([128, 2, QC], F32, tag="ot", name=f"o{qg}")
                    ew_copy(ot, vt, QP)
                    nc.sync.dma_start(out=outr[:, :, ts(qg, QC)], in_=ot)

            for qp in range(NQP):
                x0t = [None, None]
                x0q = []

                def emit_x0(pj, eT):
                    for qc in range(2):
                        if X0_DR:
                            nc.tensor.matmul(
                                x0t[qc],
                                lhsT=hT8_sb[:, 2 * pj : 2 * pj + 2, :],
                                rhs=eT[:, :, qc, :].bitcast(F8E5),
                                start=(pj == 0), stop=(pj == NKC // 2 - 1),
                                perf_mode=DR,
                            )
                        else:
                            for jj in range(2):
                                nc.tensor.matmul(
                                    x0t[qc][0:MHP, :],
                                    lhsT=hT8_sb[:, 2 * pj + jj, :],
                                    rhs=eT[:, jj, qc, :].bitcast(F8E5),
                                    start=(pj == 0 and jj == 0),
                                    stop=(pj == NKC // 2 - 1 and jj == 1),
                                )

                for kc in range(NKC):
                    ps = pl.tile([128, 2, QC], F32, tag="lg", name=f"l{qp}_{kc}")
                    for qc in range(2):
                        if LOGITS_DR:
                            nc.tensor.matmul(
                                ps[:, qc, :],
                                lhsT=g8_sb[:, :, ts(kc, KC)],
                                rhs=x8_sb[:, :, ts(2 * qp + qc, QC)].bitcast(F8E4),
                                start=True, stop=True, perf_mode=DR,
                            )
                        else:
                            for cc in range(2):
                                nc.tensor.matmul(
                                    ps[:, qc, :],
                                    lhsT=g8_sb[:, cc, ts(kc, KC)],
                                    rhs=x8_sb[:, cc, ts(2 * qp + qc, QC)].bitcast(F8E4),
                                    start=(cc == 0), stop=(cc == 1),
                                )
                    j = kc % 2
                    if j == 0:
                        eT = eTp.tile(
                            [128, 2, 2, QC], U8, tag="eT",
                            name=f"e{qp}_{kc // 2}",
                        )
                    eng = sched.pick(QP)
                    eslice = eT[:, j, :, :]
                    if DEBUG and qp == 0 and kc < 4:
                        pst = outp.tile([128, 2, QC], F32, tag="ot", name=f"dps{kc}")
                        nc.vector.tensor_copy(pst, ps)
                        nc.sync.dma_start(
                            out=dbg_ps_d[kc].rearrange("p (a b) -> p a b", a=2),
                            in_=pst,
                        )
                    bias_n = cbias_n if CONST_BIAS else ucn_sb[:, kc : kc + 1]
                    bias_s = cbias_s if CONST_BIAS else ucs_sb[:, kc : kc + 1]
                    if eng == "act":
                        nc.scalar.activation(
                            out=eslice.bitcast(F8E5), in_=ps, func=EXPF,
                            bias=bias_n, scale=AEXP_NAT,
                        )
                    else:
                        nc.vector.tensor_scalar(
                            eslice, ps, AEXP_SCH, bias_s, MULT, ADD,
                        )
                    if j == 1:
                        if DEBUG and qp == 0 and kc < 8:
                            nc.sync.dma_start(
                                out=dbg_eT_d[kc // 2],
                                in_=eT.rearrange("p a b c -> p (a b c)"),
                            )
                        x0q.append((kc // 2, eT))
                    if kc == 0 and x0t[0] is None:
                        for qc in range(2):
                            x0t[qc] = px0.tile(
                                [MHP, QC], F32, tag="x0", name=f"x0_{qp}_{qc}"
                            )
                    # flush deferred v-work from the previous qp into this
                    # qp's logits stream so the PE never waits on normalize
                    if kc == 3:
                        flush_v()
                    if len(x0q) > RUNWAY:
                        pj, peT = x0q.pop(0)
                        emit_x0(pj, peT)
                for pj, peT in x0q:
                    emit_x0(pj, peT)
                for qc in range(2):
                    pend_v.append((2 * qp + qc, x0t[qc]))
            flush_v()

    nc.compile()
    return nc


def kernel(x, wf, bf, wg, bg, wh, bh, wv, bv, gamma):
    global _cached_nc, LAST_EXEC_NS
    if _cached_nc is None:
        _cached_nc = _build()
    nc = _cached_nc

    # NOTE: device float8e4 is IEEE e4m3 (inf at 0x78, max finite 240),
    # i.e. ml_dtypes.float8_e4m3 -- NOT e4m3fn.
    E4 = ml_dtypes.float8_e4m3

    def q4(v):
        return np.clip(v, -240.0, 240.0).astype(E4).view(np.uint8)

    x = np.asarray(x, dtype=np.float32)
    wf = np.asarray(wf, dtype=np.float32)
    bf = np.asarray(bf, dtype=np.float32)
    wg = np.asarray(wg, dtype=np.float32)
    bg = np.asarray(bg, dtype=np.float32)
    wh = np.asarray(wh, dtype=np.float32)
    bh = np.asarray(bh, dtype=np.float32)
    wv = np.asarray(wv, dtype=np.float32)
    bv = np.asarray(bv, dtype=np.float32)
    g0 = float(np.asarray(gamma, dtype=np.float32).reshape(-1)[0])

    xf = np.ascontiguousarray(x.reshape(B, C, N))

    # logits weight A = wf^T wg, scaled into e4m3 range; laid out as
    # lhsT[p, cc, o, m] = (SA*A)[o*128+m, cc*128+p]
    As = (SA * (wf.T @ wg)).astype(np.float32)
    A8 = q4(np.ascontiguousarray(
        As.reshape(2, 128, 2, 128).transpose(3, 2, 0, 1)
    )).reshape(128, 512)

    # h-projection rhs: [64*wh^T | zero pad], laid out [p, cc, m]
    whu = np.zeros((C, MHP), np.float32)
    whu[:, 0:CI] = SH * wh.T
    whu8 = q4(np.ascontiguousarray(
        whu.reshape(2, 128, MHP).transpose(1, 0, 2)
    )).reshape(128, 2 * MHP)

    # wv' rows: [g0*(bv + wv@bh) | g0*wv^T/64 | 0]
    wvp = np.zeros((MH, C), np.float32)
    wvp[0, :] = g0 * (bv + wv @ bh)
    wvp[1 : CI + 1, :] = (g0 / SH) * wv.T
    wvp = np.ascontiguousarray(wvp)

    ident = np.eye(128, dtype=np.float32)
    uvec = bf @ wg  # per-key logit offset direction: u_k = uvec @ x_k

    in_maps = []
    for core in range(NCORES):
        b, half = divmod(core, 2)
        xb = xf[b]
        if half:
            xb = np.ascontiguousarray(
                np.concatenate([xb[:, NQ:], xb[:, :NQ]], axis=1)
            )
        u = (uvec @ xb).astype(np.float32)          # (N,)
        ut = u.reshape(NKC, 128).T                  # [p, kc]
        ucs = (L2E4 * (ut - CSHIFT) + 60.0).astype(np.float32)
        ucn = (ut - CSHIFT).astype(np.float32)
        in_maps.append({
            "x8": q4(xb),
            "x32": xb,
            "ucs": np.ascontiguousarray(ucs),
            "ucn": np.ascontiguousarray(ucn),
            "A8": A8, "whu8": whu8, "wvp": wvp, "ident": ident,
        })

    res = run_bass_kernel_spmd(
        nc, in_maps, list(range(NCORES)),
        trace=TRACE or bool(os.environ.get("BASS_KERNEL_TRACE")),
    )
    LAST_EXEC_NS = res.exec_time_ns
    global LAST_RES
    LAST_RES = res

    out = np.empty((B, C, N), np.float32)
    for core in range(NCORES):
        b, half = divmod(core, 2)
        out[b][:, half * NQ : (half + 1) * NQ] = res.results[core]["out"]
    return out.reshape(B, C, W, H)
